# revision 1
# baseline (speedup 1.0000x reference)
"""Cosine multihead attention on 8 Trainium2 NeuronCores.

Sharding: batch*heads across cores. Core c handles batch b = c // 4 and the
4 heads [4*(c%4), 4*(c%4)+4). Each core computes its heads' q/k/v projections
(tensor-parallel slices of in_proj), full attention for its (B,H) slice, and a
partial out-projection (rank-256 contribution). The host sums the 4 partials
per batch and adds out_proj_bias.

Schedule: the ACT engine's exp stream (128 x [128,1024] activations at
~1 elem/cycle/lane) is the critical resource. Everything else is
software-pipelined underneath it: q/k/v projections, q/k normalization and
the out-projection are emitted as PE "filler" steps interleaved into the
attention inner loop, and a deep ex-tile ring lets ACT run ahead of PV while
PE chews fillers.

Key tricks:
- q,k projected in transposed orientation (head_dim on partitions) so QK^T
  needs no transpose; 2 heads per PE pass via row tiling (K=64 at bases 0/64).
- 1/sqrt(sumsq) computed as exp(-0.5*ln(x)) on ACT: Ln and Exp share the
  natural_log_exp_and_others table set (enforced via a table-list override),
  so norms slot into the exp stream with zero table reloads. Sumsq for up to
  4 projection units batches into one [97, W] PSUM tile (row slots 0/32/64/96)
  -> one ln + one exp per group.
- 1/tau folds into the exp's per-partition bias: st = exp(-0.5 ln(ss) +
  ln(1/tau)) directly yields the combined normalizer for the k side.
- norm factors broadcast to 64-row blocks with gpsimd partition_broadcast
  (no PE matmuls, fp32-exact); softmax 1/z likewise.
- projection bias added via per-partition tensor_scalar_add during the PSUM
  drain; v bias via a prebuilt broadcast tile in the v drain add.
- softmax denominators ride as a 65th v column (ones) through PV.
"""

import os
import sys

if "/opt/trn_rl_repo" not in sys.path:
    sys.path.insert(0, "/opt/trn_rl_repo")

DEBUG = bool(os.environ.get("KERNEL_DEBUG"))

from collections import deque

import numpy as np
import ml_dtypes

import bass_rust as _bass_rust
import concourse.bass as bass
import concourse.tile as tile
from concourse import bacc, mybir
from concourse.bass_utils import run_bass_kernel_spmd
from concourse.hw_specs import get_activation_tables

S, B, E, H = 2048, 2, 1024, 16
HD = E // H            # 64
HPC = 4                # heads per core
NCORES = 8
TAU_MIN = 0.01

BF16 = ml_dtypes.bfloat16
DT_BF = mybir.dt.bfloat16
DT_F32 = mybir.dt.float32

KC_E = E // 128        # 8 contraction chunks for projections
MQ = S // 128          # 16 seq chunks of 128
NPAIR = HPC // 2       # 2 head pairs per core
AF = mybir.ActivationFunctionType

# exp bias config columns (rows 0/32/64/96 hold per-slot ln(1/tau) or 0)
CFG_MIX0, CFG_ALLK0, CFG_ALLK1, CFG_ALLQ = range(4)


def _insert_act_loads_shared(nc):
    """Drop-in for Bacc.insert_act_table_loads: keep the canonical set list
    (ids must stay aligned with act_info.json) but strip Exp/Ln from every
    set other than natural_log_exp_and_others, which genuinely contains
    both. The placement pass then settles on that one set -> a single
    table load instead of one per Ln<->Exp alternation."""
    tables = []
    for name, funcs in get_activation_tables(nc.m.arch).items():
        if name != "natural_log_exp_and_others":
            funcs = {f for f in funcs if f not in (AF.Exp, AF.Ln)}
        tables.append((name, funcs))
    _bass_rust.insert_act_table_loads(nc, tables)


def build_program():
    nc = bacc.Bacc(None)
    nc.insert_act_table_loads = lambda: _insert_act_loads_shared(nc)

    xq = nc.dram_tensor("xq_t", [E, S], DT_BF, kind="ExternalInput")
    xk = nc.dram_tensor("xk_t", [E, S], DT_BF, kind="ExternalInput")
    xv = nc.dram_tensor("xv_t", [E, S], DT_BF, kind="ExternalInput")
    wq = nc.dram_tensor("wq_t", [E, 256], DT_BF, kind="ExternalInput")
    wk = nc.dram_tensor("wk_t", [E, 256], DT_BF, kind="ExternalInput")
    wv = nc.dram_tensor("wv_t", [E, 256], DT_BF, kind="ExternalInput")
    bqc = nc.dram_tensor("bq_col", [128, 2], DT_F32, kind="ExternalInput")
    bkc = nc.dram_tensor("bk_col", [128, 2], DT_F32, kind="ExternalInput")
    bv = nc.dram_tensor("b_v", [1, 256], DT_BF, kind="ExternalInput")
    wo = nc.dram_tensor("wo_t", [256, E], DT_BF, kind="ExternalInput")
    tb_in = nc.dram_tensor("tb", [34, 4], DT_F32, kind="ExternalInput")
    sel2_in = nc.dram_tensor("sel2", [34, 128], DT_F32, kind="ExternalInput")
    outp = nc.dram_tensor("out_p", [S, E], DT_F32, kind="ExternalOutput")
    if DEBUG:
        dbg_qt = nc.dram_tensor("dbg_qt", [128, S], DT_BF, kind="ExternalOutput")
        dbg_kt = nc.dram_tensor("dbg_kt", [128, S], DT_BF, kind="ExternalOutput")
        dbg_ht = nc.dram_tensor("dbg_ht", [128, S], DT_BF, kind="ExternalOutput")
        dbg_rb = nc.dram_tensor("dbg_rb", [128, 512], DT_F32, kind="ExternalOutput")
        dbg_zbi = nc.dram_tensor("dbg_zbi", [64, 512], DT_F32, kind="ExternalOutput")
        dbg_st = nc.dram_tensor("dbg_st", [34, 512], DT_F32, kind="ExternalOutput")
        dbg_z = nc.dram_tensor("dbg_z", [1, 512], DT_F32, kind="ExternalOutput")
        dbg_n = [0]

    with tile.TileContext(nc) as tc:
        with (
            tc.tile_pool(name="consts", bufs=1) as consts,
            tc.tile_pool(name="xin", bufs=1) as xin,
            tc.tile_pool(name="wts", bufs=1) as wts,
            tc.tile_pool(name="qk", bufs=1) as qkpool,
            tc.tile_pool(name="vsb", bufs=1) as vpool,
            tc.tile_pool(name="sqp", bufs=2) as sqp,
            tc.tile_pool(name="stp", bufs=1) as stp,
            tc.tile_pool(name="exp", bufs=14) as expool,
            tc.tile_pool(name="work", bufs=2) as work,
            tc.tile_pool(name="ltp", bufs=1) as ltp,
            tc.tile_pool(name="rbp", bufs=2) as rbp,
            tc.tile_pool(name="zz", bufs=2) as zz,
            tc.tile_pool(name="outs", bufs=2) as outs,
            tc.tile_pool(name="ps_sc", bufs=2, space="PSUM") as ps_sc,
            tc.tile_pool(name="ps_o", bufs=2, space="PSUM") as ps_o,
            tc.tile_pool(name="ps_tr", bufs=1, space="PSUM") as ps_tr,
        ):
            # ---- constants ----------------------------------------------
            tb_sb = consts.tile([34, 4], DT_F32, tag="tb")
            nc.sync.dma_start(out=tb_sb, in_=tb_in[:, :])
            bq_sb = consts.tile([128, 2], DT_F32, tag="bq")
            nc.sync.dma_start(out=bq_sb, in_=bqc[:, :])
            bk_sb = consts.tile([128, 2], DT_F32, tag="bk")
            nc.sync.dma_start(out=bk_sb, in_=bkc[:, :])
            bv_sb = consts.tile([1, 256], DT_BF, tag="bv")
            nc.sync.dma_start(out=bv_sb, in_=bv[:, :])

            ones_row = consts.tile([1, 128], DT_BF, tag="ones_row")
            nc.vector.memset(ones_row, 1.0)
            # sumsq selectors: A -> out rows 0 (parts 0:64) / 32 (64:128),
            #                  B -> out rows 64 / 96
            hselA = consts.tile([128, 34], DT_BF, tag="hselA")
            nc.vector.memset(hselA, 0.0)
            nc.vector.memset(hselA[0:64, 0:1], 1.0)
            nc.vector.memset(hselA[64:128, 1:2], 1.0)
            hselB = consts.tile([128, 34], DT_BF, tag="hselB")
            nc.vector.memset(hselB, 0.0)
            nc.vector.memset(hselB[0:64, 32:33], 1.0)
            nc.vector.memset(hselB[64:128, 33:34], 1.0)
            sel2 = consts.tile([34, 128], DT_F32, tag="sel2")
            nc.sync.dma_start(out=sel2, in_=sel2_in[:, :])

            # ---- weights ------------------------------------------------
            wq_sb = wts.tile([128, KC_E, 256], DT_BF, tag="wq")
            wk_sb = wts.tile([128, KC_E, 256], DT_BF, tag="wk")
            wv_sb = wts.tile([128, KC_E, 256], DT_BF, tag="wv")
            for c in range(KC_E):
                nc.sync.dma_start(out=wv_sb[:, c, :], in_=wv[c * 128:(c + 1) * 128, :])
            for c in range(KC_E):
                nc.sync.dma_start(out=wq_sb[:, c, :], in_=wq[c * 128:(c + 1) * 128, :])
                nc.scalar.dma_start(out=wk_sb[:, c, :], in_=wk[c * 128:(c + 1) * 128, :])

            # ---- activations, need-ordered column blocks ----------------
            # SP ring: xq; ACT ring: xk; SWDGE (gpsimd): xv
            xq_sb = xin.tile([128, KC_E, S], DT_BF, tag="xq")
            xk_sb = xin.tile([128, KC_E, S], DT_BF, tag="xk")
            xv_sb = xin.tile([128, KC_E, S], DT_BF, tag="xv")

            def dma_xblock(eng, dst, src, c, c0, c1):
                eng.dma_start(out=dst[:, c, c0:c1], in_=src[c * 128:(c + 1) * 128, c0:c1])

            for c in range(KC_E):
                dma_xblock(nc.sync, xv_sb, xv, c, 0, 512)
            for c in range(KC_E):
                dma_xblock(nc.scalar, xk_sb, xk, c, 0, 1024)
            for c in range(KC_E):
                dma_xblock(nc.sync, xq_sb, xq, c, 0, 512)
            for c in range(KC_E):
                dma_xblock(nc.sync, xv_sb, xv, c, 512, 1024)
            for c in range(KC_E):
                dma_xblock(nc.scalar, xk_sb, xk, c, 1024, 2048)
            for c in range(KC_E):
                dma_xblock(nc.sync, xq_sb, xq, c, 512, 1024)
            for c in range(KC_E):
                dma_xblock(nc.scalar, xv_sb, xv, c, 1536, 2048)
            for c in range(KC_E):
                dma_xblock(nc.sync, xv_sb, xv, c, 1024, 1536)
            for c in range(KC_E):
                dma_xblock(nc.sync, xq_sb, xq, c, 1024, 2048)
            wo_sb = wts.tile([128, 2, E], DT_BF, tag="wo")
            for c in range(2):
                nc.sync.dma_start(out=wo_sb[:, c, :], in_=wo[c * 128:(c + 1) * 128, :])

            qt = [qkpool.tile([128, S], DT_BF, tag=f"qt{p}", name=f"qt{p}")
                  for p in range(NPAIR)]
            kt = [qkpool.tile([128, S], DT_BF, tag=f"kt{p}", name=f"kt{p}")
                  for p in range(NPAIR)]
            heads_t = [qkpool.tile([128, S], DT_BF, tag=f"ht{p}", name=f"ht{p}")
                       for p in range(NPAIR)]

            v_sb = vpool.tile([128, MQ, HPC, HD + 1], DT_BF, tag="v")
            nc.vector.memset(v_sb[:, :, :, HD:HD + 1], 1.0)

            # =============== unit emitters ===============================
            # A projection "unit" computes a [128, 512] block of qt/kt.
            # Units are (which, mc, u): which 'q'/'k', mc head-pair, u block.

            def proj_unit_steps(which, mc, u, get_sq2, half):
                dst = (qt if which == "q" else kt)[mc]
                w_sb = wq_sb if which == "q" else wk_sb
                x_sb = xq_sb if which == "q" else xk_sb
                b_sb = bq_sb if which == "q" else bk_sb
                sl = slice(u * 512, (u + 1) * 512)
                st8 = {}

                def mk_mm(c):
                    def go():
                        if c == 0:
                            st8["pp"] = ps_tr.tile([128, 512], DT_F32,
                                                   tag="tr", name="pp_t")
                        nc.tensor.matmul(
                            st8["pp"],
                            lhsT=w_sb[:, c, mc * 128:(mc + 1) * 128],
                            rhs=x_sb[:, c, sl],
                            start=(c == 0),
                            stop=(c == KC_E - 1),
                        )
                    return go

                def drain():
                    nc.vector.tensor_scalar_add(
                        dst[:, sl], st8["pp"], b_sb[:, mc:mc + 1])
                    nc.vector.tensor_mul(
                        get_sq2()[:, half * 512:(half + 1) * 512],
                        dst[:, sl], dst[:, sl])

                steps = []
                for c in range(0, KC_E, 2):
                    cc = c
                    steps.append((430,
                                  lambda cc=cc: (mk_mm(cc)(), mk_mm(cc + 1)()),
                                  "start" if c == 0 else "mid"))
                steps.append((900, drain, "end"))
                return steps

            def norm_group_steps(units, cfg, get_a, get_b, width):
                """units: 2 or 4 (which, mc, u). Unit slot i: row pair
                (0,32) for i%2==0 via hselA / (64,96) for i%2==1 via hselB;
                col half i//2. Squares for row-pair-A slots live in get_a()
                (col half per slot), row-pair-B slots in get_b().
                One ln + one exp(bias=ln(1/tau), scale=-0.5) per group, then
                per-unit partition_broadcast + normalize muls."""
                st8 = {}
                nh = width // 512
                solo = len(units) == 1
                nrow = 2 if solo else 34

                def mk_mm_ss(ch):
                    def go():
                        if ch == 0:
                            st8["ss"] = ps_tr.tile([nrow, width], DT_F32,
                                                   tag="tr", name="ss_t")
                        csl = slice(ch * 512, (ch + 1) * 512)
                        nc.tensor.matmul(st8["ss"][:, csl],
                                         lhsT=hselA[:, 0:nrow],
                                         rhs=get_a()[:, csl],
                                         start=True, stop=solo)
                        if not solo:
                            nc.tensor.matmul(st8["ss"][:, csl],
                                             lhsT=hselB[:, 0:nrow],
                                             rhs=get_b()[:, csl],
                                             start=False, stop=True)
                    return go

                def act_ln():
                    lt = ltp.tile([nrow, width], DT_F32, tag="lt", name="lt_t")
                    st8["lt"] = lt
                    nc.scalar.activation(lt, st8["ss"], AF.Ln)

                def act_exp():
                    st = stp.tile([nrow, width], DT_F32, tag="st", name="st_t")
                    st8["st"] = st
                    nc.scalar.activation(st, st8["lt"], AF.Exp,
                                         tb_sb[0:nrow, cfg:cfg + 1], -0.5)

                def mk_bcmul(i):
                    which, mc, u = units[i]
                    dst = (qt if which == "q" else kt)[mc]
                    r0 = 32 * (i % 2)
                    c0 = 512 * (i // 2)
                    sl = slice(u * 512, (u + 1) * 512)

                    def go():
                        rb = ps_tr.tile([128, 512], DT_F32, tag="tr",
                                        name="rb_t")
                        nc.tensor.matmul(
                            rb, lhsT=sel2[r0:r0 + 2, :],
                            rhs=st8["st"][r0:r0 + 2, c0:c0 + 512],
                            start=True, stop=True)
                        nc.vector.tensor_mul(dst[:, sl], dst[:, sl], rb)
                        if DEBUG and dbg_n[0] == 0:
                            dbg_n[0] = 1
                            rbc = outs.tile([128, 512], DT_F32, tag="ob",
                                            name="rbc_t")
                            nc.vector.tensor_copy(rbc, rb)
                            nc.sync.dma_start(out=dbg_rb[:, :], in_=rbc)
                            nc.sync.dma_start(
                                out=dbg_st[0:st8["st"].shape[0], 0:512],
                                in_=st8["st"][:, 0:512])
                    return go

                steps = [(215, mk_mm_ss(ch), "start" if ch == 0 else "mid")
                         for ch in range(nh)]
                steps.append((450, act_ln, "end"))
                steps.append((450, act_exp, "safe"))
                for i in range(len(units)):
                    steps.append((650, mk_bcmul(i), "safe", units[i]))
                return steps

            def v_unit_steps(m):
                st8 = {}

                def mk_mm(c):
                    def go():
                        if c == 0:
                            st8["vp"] = ps_tr.tile([128, 256], DT_F32,
                                                   tag="tr", name="vp_t")
                        nc.tensor.matmul(
                            st8["vp"],
                            lhsT=xv_sb[:, c, m * 128:(m + 1) * 128],
                            rhs=wv_sb[:, c, :],
                            start=(c == 0),
                            stop=False,
                        )
                    return go

                def bias_drain():
                    nc.tensor.matmul(
                        st8["vp"],
                        lhsT=ones_row[0:1, 0:128],
                        rhs=bv_sb[0:1, :],
                        start=False, stop=True)
                    nc.vector.tensor_copy(
                        out=v_sb[:, m, :, 0:HD],
                        in_=st8["vp"].rearrange("p (h d) -> p h d", h=HPC))

                steps = []
                for c in range(0, KC_E, 2):
                    cc = c
                    steps.append((230,
                                  lambda cc=cc: (mk_mm(cc)(), mk_mm(cc + 1)()),
                                  "start" if c == 0 else "mid"))
                steps.append((600, bias_drain, "end", ("v", m)))
                return steps

            def outproj_steps(m, n2):
                sl_n = slice(n2 * 512, (n2 + 1) * 512)
                st8 = {}

                def mms():
                    st8["op"] = ps_tr.tile([128, 512], DT_F32, tag="tr",
                                           name="op_t")
                    nc.tensor.matmul(st8["op"],
                                     lhsT=heads_t[0][:, m * 128:(m + 1) * 128],
                                     rhs=wo_sb[:, 0, sl_n],
                                     start=True, stop=False)
                    nc.tensor.matmul(st8["op"],
                                     lhsT=heads_t[1][:, m * 128:(m + 1) * 128],
                                     rhs=wo_sb[:, 1, sl_n],
                                     start=False, stop=True)

                def drain():
                    ob = outs.tile([128, 512], DT_F32, tag="ob", name="ob_t")
                    nc.vector.tensor_copy(ob, st8["op"])
                    nc.sync.dma_start(out=outp[m * 128:(m + 1) * 128, sl_n],
                                      in_=ob)
                return [(430, mms, "start"), (700, drain, "end")]

            # =============== filler machinery ============================
            # Steps are (cost_ns, fn, kind): "start" acquires the ps_tr
            # ring, "end" releases it, "mid" holds, "safe" self-contained.
            # The qb-end chain must never alloc ps_tr while a filler unit
            # holds it with its releasing step un-emitted (deadlock), hence
            # flush_unit().
            fillers = deque()
            carry = [0.0]
            tr_open = [False]
            emitted = set()

            def _run_step(step):
                cost, fn, kind = step[0], step[1], step[2]
                fn()
                if kind == "start":
                    tr_open[0] = True
                elif kind == "end":
                    tr_open[0] = False
                if len(step) > 3:
                    emitted.add(step[3])

            def pump(budget_ns):
                budget = budget_ns + carry[0]
                while fillers and budget >= fillers[0][0]:
                    budget -= fillers[0][0]
                    _run_step(fillers.popleft())
                carry[0] = min(budget, 800.0)

            def flush_unit():
                while fillers and tr_open[0]:
                    _run_step(fillers.popleft())

            def gate(label):
                """Emission-order dependency: force-emit filler steps until
                the unit producing `label` has been emitted. Without this, a
                consumer emitted before its producer has no dependency edge
                (Tile resolves deps in emission order) -> races."""
                while label not in emitted:
                    assert fillers, f"gate on {label} but fillers empty"
                    _run_step(fillers.popleft())

            def add_unit_group(units_seq, cfg):
                """Steps for a group of proj units + their joint norm. sq2
                staging tiles are allocated lazily at first use so pool ring
                order matches instruction emission order."""
                steps = []
                n = len(units_seq)
                assert n in (1, 2, 4)
                width = 512 * max(1, n // 2)
                hold = {}

                def getter(key):
                    def get():
                        if key not in hold:
                            hold[key] = sqp.tile([128, width], DT_BF,
                                                 tag="sq2", name=f"sq2{key}")
                        return hold[key]
                    return get

                get_a, get_b = getter("a"), getter("b")
                for i, (which, mc, u) in enumerate(units_seq):
                    tgt = get_a if i % 2 == 0 else get_b
                    steps += proj_unit_steps(which, mc, u, tgt, i // 2)
                steps += norm_group_steps(units_seq, cfg, get_a, get_b, width)
                return steps

            def run_steps(steps):
                for step in steps:
                    _run_step(step)

            # =============== head phase ==================================
            for m in range(2):
                run_steps(v_unit_steps(m))
            run_steps(add_unit_group([("q", 0, 0), ("k", 0, 0)], CFG_MIX0))
            run_steps(add_unit_group([("q", 0, 1), ("k", 0, 1)], CFG_MIX0))
            run_steps(v_unit_steps(2))
            run_steps(v_unit_steps(3))

            # =============== filler stream (need-ordered) ================
            for m in range(4, 8):
                for st_ in v_unit_steps(m):
                    fillers.append(st_)
            for st_ in add_unit_group([("k", 0, 2)], CFG_ALLK0):
                fillers.append(st_)
            for m in range(8, 12):
                for st_ in v_unit_steps(m):
                    fillers.append(st_)
            for st_ in add_unit_group([("k", 0, 3)], CFG_ALLK0):
                fillers.append(st_)
            for m in range(12, MQ):
                for st_ in v_unit_steps(m):
                    fillers.append(st_)
            for st_ in add_unit_group([("q", 0, 2), ("q", 0, 3)], CFG_ALLQ):
                fillers.append(st_)
            for st_ in add_unit_group([("k", 1, 0), ("k", 1, 1),
                                       ("k", 1, 2), ("k", 1, 3)], CFG_ALLK1):
                fillers.append(st_)
            for st_ in add_unit_group([("q", 1, 0), ("q", 1, 1),
                                       ("q", 1, 2), ("q", 1, 3)], CFG_ALLQ):
                fillers.append(st_)

            # =============== attention ===================================
            def attention_pair(p, budget, after_qb=None):
                for qb in range(4):
                    sl_q = slice(qb * 512, (qb + 1) * 512)
                    o0 = ps_o.tile([128, 512], DT_F32, tag="o", name="o0_t")
                    o1 = ps_o.tile([128, 512], DT_F32, tag="o", name="o1_t")
                    for kc in range(MQ):
                        gate(("k", p, kc // 4))
                        gate(("q", p, qb))
                        gate(("v", kc))
                        sc = ps_sc.tile([128, 1024], DT_F32, tag="sc",
                                        name="sc_t")
                        nc.tensor.matmul(
                            sc[:, 0:512],
                            lhsT=kt[p][0:64, kc * 128:(kc + 1) * 128],
                            rhs=qt[p][0:64, sl_q],
                            start=True, stop=True)
                        nc.tensor.matmul(
                            sc[:, 512:1024],
                            lhsT=kt[p][64:128, kc * 128:(kc + 1) * 128],
                            rhs=qt[p][64:128, sl_q],
                            start=True, stop=True)
                        ex = expool.tile([128, 1024], DT_BF, tag="ex",
                                         name="ex_t")
                        nc.scalar.activation(ex, sc, AF.Exp)
                        nc.tensor.matmul(
                            o0[0:65, :],
                            lhsT=v_sb[:, kc, 2 * p, :],
                            rhs=ex[:, 0:512],
                            start=(kc == 0), stop=(kc == MQ - 1))
                        nc.tensor.matmul(
                            o1[0:65, :],
                            lhsT=v_sb[:, kc, 2 * p + 1, :],
                            rhs=ex[:, 512:1024],
                            start=(kc == 0), stop=(kc == MQ - 1))
                        pump(budget)
                    # softmax normalize + write heads_t; free ps_tr first
                    flush_unit()
                    for hl, o in ((0, o0), (1, o1)):
                        zs = zz.tile([1, 512], DT_F32, tag="zi", name="zs_t")
                        nc.vector.tensor_copy(zs, o[64:65, :])
                        zb = ps_tr.tile([64, 512], DT_F32, tag="tr",
                                        name="zb_t")
                        nc.tensor.matmul(zb, lhsT=sel2[0:1, 0:64], rhs=zs,
                                         start=True, stop=True)
                        zbi = zz.tile([64, 512], DT_F32, tag="zbi",
                                      name="zbi_t")
                        nc.vector.reciprocal_approx_fast(out=zbi, in_=zb)
                        if DEBUG and p == 0 and qb == 0 and hl == 0:
                            nc.sync.dma_start(out=dbg_zbi[:, :], in_=zbi)
                            zc = zz.tile([1, 512], DT_F32, tag="zi",
                                         name="zc_t")
                            nc.vector.tensor_copy(zc, o[64:65, :])
                            nc.sync.dma_start(out=dbg_z[:, :], in_=zc)
                        if hl == 0:
                            nc.vector.tensor_mul(
                                heads_t[p][0:64, sl_q], o[0:64, :], zbi)
                        else:
                            t2 = work.tile([64, 512], DT_BF, tag="t2",
                                           name="t2_t")
                            nc.vector.tensor_mul(t2, o[0:64, :], zbi)
                            nc.sync.dma_start(
                                out=heads_t[p][64:128, sl_q], in_=t2)
                    if after_qb is not None:
                        after_qb(qb)

            attention_pair(0, budget=520)

            def pair1_after_qb(qb):
                for m in range(qb * 4, qb * 4 + 4):
                    for n2 in range(2):
                        for st_ in outproj_steps(m, n2):
                            fillers.append(st_)

            attention_pair(1, budget=470, after_qb=pair1_after_qb)

            # =============== tail ========================================
            while fillers:
                _run_step(fillers.popleft())
            if DEBUG:
                nc.sync.dma_start(out=dbg_qt[:, :], in_=qt[0])
                nc.sync.dma_start(out=dbg_kt[:, :], in_=kt[0])
                nc.sync.dma_start(out=dbg_ht[:, :], in_=heads_t[0])

    nc.compile()
    return nc


_CACHE = {}


def _get_program():
    if "nc" not in _CACHE:
        _CACHE["nc"] = build_program()
    return _CACHE["nc"]


def make_in_maps(query, key, value, in_proj_weight, in_proj_bias,
                 out_proj_weight, out_proj_bias, tau):
    query = np.asarray(query, np.float32)
    key = np.asarray(key, np.float32)
    value = np.asarray(value, np.float32)
    W = np.asarray(in_proj_weight, np.float32)
    bias = np.asarray(in_proj_bias, np.float32)
    Wo = np.asarray(out_proj_weight, np.float32)
    tau_c = np.maximum(np.asarray(tau, np.float32).reshape(H), TAU_MIN)

    xT = {}
    for b in range(B):
        xT["q", b] = np.ascontiguousarray(query[:, b, :].T).astype(BF16)
        xT["k", b] = np.ascontiguousarray(key[:, b, :].T).astype(BF16)
        xT["v", b] = np.ascontiguousarray(value[:, b, :].T).astype(BF16)

    in_maps = []
    for c in range(NCORES):
        b = c // 4
        h0 = HPC * (c % 4)
        rows = slice(h0 * HD, (h0 + HPC) * HD)
        rows_k = slice(E + h0 * HD, E + (h0 + HPC) * HD)
        rows_v = slice(2 * E + h0 * HD, 2 * E + (h0 + HPC) * HD)

        # exp bias configs [34, 4]: ln(1/tau) per row slot.
        # rows {0,1} = "A" slot pair (even, odd head), {32,33} = "B".
        ltau = np.log(1.0 / tau_c)
        tb = np.zeros((34, 4), np.float32)
        tb[32, CFG_MIX0] = ltau[h0 + 0]
        tb[33, CFG_MIX0] = ltau[h0 + 1]
        tb[0, CFG_ALLK0] = ltau[h0 + 0]
        tb[1, CFG_ALLK0] = ltau[h0 + 1]
        tb[32, CFG_ALLK0] = ltau[h0 + 0]
        tb[33, CFG_ALLK0] = ltau[h0 + 1]
        tb[0, CFG_ALLK1] = ltau[h0 + 2]
        tb[1, CFG_ALLK1] = ltau[h0 + 3]
        tb[32, CFG_ALLK1] = ltau[h0 + 2]
        tb[33, CFG_ALLK1] = ltau[h0 + 3]
        # CFG_ALLQ stays zero
        sel2 = np.zeros((34, 128), np.float32)
        for r0 in (0, 32):
            sel2[r0 + 0, 0:64] = 1.0
            sel2[r0 + 1, 64:128] = 1.0

        bq_col = np.zeros((128, 2), np.float32)
        bk_col = np.zeros((128, 2), np.float32)
        for mc in range(NPAIR):
            bq_col[:, mc] = bias[rows][mc * 128:(mc + 1) * 128]
            bk_col[:, mc] = bias[rows_k][mc * 128:(mc + 1) * 128]

        in_maps.append({
            "xq_t": xT["q", b],
            "xk_t": xT["k", b],
            "xv_t": xT["v", b],
            "wq_t": np.ascontiguousarray(W[rows, :].T).astype(BF16),
            "wk_t": np.ascontiguousarray(W[rows_k, :].T).astype(BF16),
            "wv_t": np.ascontiguousarray(W[rows_v, :].T).astype(BF16),
            "bq_col": bq_col,
            "bk_col": bk_col,
            "b_v": bias[rows_v].reshape(1, 256).astype(BF16),
            "wo_t": np.ascontiguousarray(Wo[:, rows].T).astype(BF16),
            "tb": tb,
            "sel2": sel2,
        })
    return in_maps


def assemble_out(results, out_proj_bias):
    bo = np.asarray(out_proj_bias, np.float32)
    out = np.zeros((S, B, E), np.float32)
    for c in range(NCORES):
        out[:, c // 4, :] += results[c]["out_p"]
    out += bo[None, None, :]
    return out


def kernel(query, key, value, in_proj_weight, in_proj_bias,
           out_proj_weight, out_proj_bias, tau):
    nc = _get_program()
    in_maps = make_in_maps(query, key, value, in_proj_weight, in_proj_bias,
                           out_proj_weight, out_proj_bias, tau)
    res = run_bass_kernel_spmd(nc, in_maps, core_ids=list(range(NCORES)))
    return assemble_out(res.results, out_proj_bias)


if __name__ == "__main__":
    import reference

    inputs = {k: np.asarray(v) for k, v in reference.setup_inputs().items()}
    out = kernel(**inputs)
    print("out shape", out.shape, out.dtype)



# revision 5
# speedup vs baseline: 1.3917x; 1.3917x over previous
"""Cosine multihead attention on 8 Trainium2 NeuronCores.

Sharding: batch*heads across cores. Core c handles batch b = c // 4 and the
4 heads [4*(c%4), 4*(c%4)+4). Each core computes its heads' q/k/v projections
(tensor-parallel slices of in_proj), full attention for its (B,H) slice, and a
partial out-projection (rank-256 contribution). The host sums the 4 partials
per batch and adds out_proj_bias (partials shipped bf16, summed fp32).

Schedule: the attention inner loop is software-pipelined by one iteration —
the PE emits QK(kc) then PV(kc-1), so the exp(kc) latency on ACT is hidden
behind PV(kc-1) + filler work instead of stalling the PE queue (PV(kc)
directly behind QK(kc) exposed the full ACT latency every iteration and let
the PE HAM throttle to 1.2 GHz). Projections and the out-projection are
emitted as PE "filler" steps pumped into the loop at a per-iteration budget.

Key tricks:
- q,k projected in transposed orientation (head_dim on partitions) so QK^T
  needs no transpose; 2 heads per PE pass via row tiling (K=64 at bases 0/64).
- 1/sqrt(sumsq) computed as exp(-0.5*ln(x)) on ACT: Ln and Exp share the
  natural_log_exp_and_others table set (enforced via a table-list override),
  so norms slot into the exp stream with zero table reloads. Sumsq for up to
  4 projection units batches into one [97, W] PSUM tile (row slots 0/32/64/96)
  -> one ln + one exp per group.
- 1/tau folds into the exp's per-partition bias: st = exp(-0.5 ln(ss) +
  ln(1/tau)) directly yields the combined normalizer for the k side.
- all broadcast matmuls (norm factors, softmax 1/z) run in bf16 (fp32 lhsT
  would force the PE into the 2-pass LOW_HIGH fp32 mode, ~4x slower).
- projection bias added via per-partition tensor_scalar_add during the PSUM
  drain; v bias via a prebuilt broadcast tile in the v drain add.
- softmax denominators ride as a 65th v column (ones) through PV.
- inputs DMA'd with 3D access patterns (one trigger per 1MB column block)
  spread across the sync/scalar/vector/gpsimd queues, need-ordered.
"""

import os
import sys

if "/opt/trn_rl_repo" not in sys.path:
    sys.path.insert(0, "/opt/trn_rl_repo")

from collections import deque

import numpy as np
import ml_dtypes

import bass_rust as _bass_rust
import concourse.bass as bass
import concourse.tile as tile
from concourse import bacc, mybir
from concourse.bass_utils import run_bass_kernel_spmd
from concourse.hw_specs import get_activation_tables

S, B, E, H = 2048, 2, 1024, 16
HD = E // H            # 64
HPC = 4                # heads per core
NCORES = 8
TAU_MIN = 0.01

BF16 = ml_dtypes.bfloat16
DT_BF = mybir.dt.bfloat16
DT_F32 = mybir.dt.float32

KC_E = E // 128        # 8 contraction chunks for projections
MQ = S // 128          # 16 seq chunks of 128
NPAIR = HPC // 2       # 2 head pairs per core
AF = mybir.ActivationFunctionType

# exp bias config columns (rows 0/32/64/96 hold per-slot ln(1/tau) or 0)
CFG_MIX0, CFG_ALLK0, CFG_ALLK1, CFG_ALLQ = range(4)


def _insert_act_loads_shared(nc):
    """Drop-in for Bacc.insert_act_table_loads: keep the canonical set list
    (ids must stay aligned with act_info.json) but strip Exp/Ln from every
    set other than natural_log_exp_and_others, which genuinely contains
    both. The placement pass then settles on that one set -> a single
    table load instead of one per Ln<->Exp alternation."""
    tables = []
    for name, funcs in get_activation_tables(nc.m.arch).items():
        if name != "natural_log_exp_and_others":
            funcs = {f for f in funcs if f not in (AF.Exp, AF.Ln)}
        tables.append((name, funcs))
    _bass_rust.insert_act_table_loads(nc, tables)


def build_program():
    nc = bacc.Bacc(None)
    nc.insert_act_table_loads = lambda: _insert_act_loads_shared(nc)

    xq = nc.dram_tensor("xq_t", [KC_E, 128, S], DT_BF, kind="ExternalInput")
    xk = nc.dram_tensor("xk_t", [KC_E, 128, S], DT_BF, kind="ExternalInput")
    xv = nc.dram_tensor("xv_t", [KC_E, 128, S], DT_BF, kind="ExternalInput")
    wq = nc.dram_tensor("wq_t", [KC_E, 128, 256], DT_BF, kind="ExternalInput")
    wk = nc.dram_tensor("wk_t", [KC_E, 128, 256], DT_BF, kind="ExternalInput")
    wv = nc.dram_tensor("wv_t", [KC_E, 128, 256], DT_BF, kind="ExternalInput")
    bqc = nc.dram_tensor("bq_col", [128, 2], DT_F32, kind="ExternalInput")
    bkc = nc.dram_tensor("bk_col", [128, 2], DT_F32, kind="ExternalInput")
    bv = nc.dram_tensor("b_v", [1, 256], DT_BF, kind="ExternalInput")
    wo = nc.dram_tensor("wo_t", [2, 128, E], DT_BF, kind="ExternalInput")
    tb_in = nc.dram_tensor("tb", [34, 4], DT_F32, kind="ExternalInput")
    sel2_in = nc.dram_tensor("sel2", [34, 128], DT_BF, kind="ExternalInput")
    outp = nc.dram_tensor("out_p", [S, E], DT_BF, kind="ExternalOutput")

    with tile.TileContext(nc) as tc:
        with (
            tc.tile_pool(name="consts", bufs=1) as consts,
            tc.tile_pool(name="xin", bufs=1) as xin,
            tc.tile_pool(name="wts", bufs=1) as wts,
            tc.tile_pool(name="qk", bufs=1) as qkpool,
            tc.tile_pool(name="vsb", bufs=1) as vpool,
            tc.tile_pool(name="sqp", bufs=2) as sqp,
            tc.tile_pool(name="stp", bufs=1) as stp,
            tc.tile_pool(name="exp", bufs=14) as expool,
            tc.tile_pool(name="work", bufs=2) as work,
            tc.tile_pool(name="ltp", bufs=1) as ltp,
            tc.tile_pool(name="zz", bufs=2) as zz,
            tc.tile_pool(name="outs", bufs=2) as outs,
            tc.tile_pool(name="ps_sc", bufs=2, space="PSUM") as ps_sc,
            tc.tile_pool(name="ps_o", bufs=2, space="PSUM") as ps_o,
            tc.tile_pool(name="ps_tr", bufs=1, space="PSUM") as ps_tr,
            tc.tile_pool(name="ps_zb", bufs=1, space="PSUM") as ps_zb,
        ):
            # ---- constants ----------------------------------------------
            tb_sb = consts.tile([34, 4], DT_F32, tag="tb")
            nc.sync.dma_start(out=tb_sb, in_=tb_in[:, :])
            bq_sb = consts.tile([128, 2], DT_F32, tag="bq")
            nc.sync.dma_start(out=bq_sb, in_=bqc[:, :])
            bk_sb = consts.tile([128, 2], DT_F32, tag="bk")
            nc.sync.dma_start(out=bk_sb, in_=bkc[:, :])
            bv_sb = consts.tile([1, 256], DT_BF, tag="bv")
            nc.sync.dma_start(out=bv_sb, in_=bv[:, :])
            sel2 = consts.tile([34, 128], DT_BF, tag="sel2")
            nc.sync.dma_start(out=sel2, in_=sel2_in[:, :])

            ones_row = consts.tile([1, 128], DT_BF, tag="ones_row")
            nc.vector.memset(ones_row, 1.0)
            # sumsq selectors: A -> out rows 0 (parts 0:64) / 1 (64:128),
            #                  B -> out rows 32 / 33
            hselA = consts.tile([128, 34], DT_BF, tag="hselA")
            nc.vector.memset(hselA, 0.0)
            nc.vector.memset(hselA[0:64, 0:1], 1.0)
            nc.vector.memset(hselA[64:128, 1:2], 1.0)
            hselB = consts.tile([128, 34], DT_BF, tag="hselB")
            nc.vector.memset(hselB, 0.0)
            nc.vector.memset(hselB[0:64, 32:33], 1.0)
            nc.vector.memset(hselB[64:128, 33:34], 1.0)

            # ---- weights (one 3D DMA each) ------------------------------
            wq_sb = wts.tile([128, KC_E, 256], DT_BF, tag="wq")
            wk_sb = wts.tile([128, KC_E, 256], DT_BF, tag="wk")
            wv_sb = wts.tile([128, KC_E, 256], DT_BF, tag="wv")
            nc.gpsimd.dma_start(
                out=wv_sb[:, :, :], in_=wv[:, :, :].rearrange("c p n -> p c n"))
            nc.sync.dma_start(
                out=wq_sb[:, :, :], in_=wq[:, :, :].rearrange("c p n -> p c n"))
            nc.scalar.dma_start(
                out=wk_sb[:, :, :], in_=wk[:, :, :].rearrange("c p n -> p c n"))

            # ---- activations, need-ordered column blocks ----------------
            xq_sb = xin.tile([128, KC_E, S], DT_BF, tag="xq")
            xk_sb = xin.tile([128, KC_E, S], DT_BF, tag="xk")
            xv_sb = xin.tile([128, KC_E, S], DT_BF, tag="xv")

            def ldx(eng, dst, src, c0, c1):
                eng.dma_start(out=dst[:, :, c0:c1],
                              in_=src[:, :, c0:c1].rearrange("c p s -> p c s"))

            ldx(nc.scalar, xk_sb, xk, 0, 1024)
            ldx(nc.sync, xq_sb, xq, 0, 512)
            ldx(nc.gpsimd, xv_sb, xv, 0, 512)
            ldx(nc.scalar, xk_sb, xk, 1024, 2048)
            ldx(nc.gpsimd, xv_sb, xv, 512, 1024)
            ldx(nc.sync, xq_sb, xq, 512, 1024)
            ldx(nc.gpsimd, xv_sb, xv, 1024, 1536)
            ldx(nc.gpsimd, xv_sb, xv, 1536, 2048)
            ldx(nc.sync, xq_sb, xq, 1024, 2048)
            wo_sb = wts.tile([128, 2, E], DT_BF, tag="wo")
            nc.gpsimd.dma_start(
                out=wo_sb[:, :, :], in_=wo[:, :, :].rearrange("c p n -> p c n"))

            qt = [qkpool.tile([128, S], DT_BF, tag=f"qt{p}", name=f"qt{p}")
                  for p in range(NPAIR)]
            kt = [qkpool.tile([128, S], DT_BF, tag=f"kt{p}", name=f"kt{p}")
                  for p in range(NPAIR)]
            heads_t = [qkpool.tile([128, S], DT_BF, tag=f"ht{p}", name=f"ht{p}")
                       for p in range(NPAIR)]

            v_sb = vpool.tile([128, MQ, HPC, HD + 1], DT_BF, tag="v")
            nc.vector.memset(v_sb[:, :, :, HD:HD + 1], 1.0)

            # =============== unit emitters ===============================
            # A projection "unit" computes a [128, 512] block of qt/kt.
            # Units are (which, mc, u): which 'q'/'k', mc head-pair, u block.

            def proj_unit_steps(which, mc, u, get_sq2, half):
                dst = (qt if which == "q" else kt)[mc]
                w_sb = wq_sb if which == "q" else wk_sb
                x_sb = xq_sb if which == "q" else xk_sb
                b_sb = bq_sb if which == "q" else bk_sb
                sl = slice(u * 512, (u + 1) * 512)
                st8 = {}

                def mk_mm(c):
                    def go():
                        if c == 0:
                            st8["pp"] = ps_tr.tile([128, 512], DT_F32,
                                                   tag="tr", name="pp_t")
                        nc.tensor.matmul(
                            st8["pp"],
                            lhsT=w_sb[:, c, mc * 128:(mc + 1) * 128],
                            rhs=x_sb[:, c, sl],
                            start=(c == 0),
                            stop=(c == KC_E - 1),
                        )
                    return go

                def drain():
                    nc.vector.tensor_scalar_add(
                        dst[:, sl], st8["pp"], b_sb[:, mc:mc + 1])
                    nc.vector.tensor_mul(
                        get_sq2()[:, half * 512:(half + 1) * 512],
                        dst[:, sl], dst[:, sl])

                steps = []
                for c in range(0, KC_E, 2):
                    cc = c
                    steps.append((430,
                                  lambda cc=cc: (mk_mm(cc)(), mk_mm(cc + 1)()),
                                  "start" if c == 0 else "mid"))
                steps.append((900, drain, "end"))
                return steps

            def norm_group_steps(units, cfg, get_a, get_b, width):
                """units: 1, 2 or 4 (which, mc, u). Unit slot i: row pair
                (0,1) for i%2==0 via hselA / (32,33) for i%2==1 via hselB;
                col half i//2. One ln + one exp(bias=ln(1/tau), scale=-0.5)
                per group, then per-unit bf16 broadcast-matmul + normalize
                muls."""
                st8 = {}
                nh = width // 512
                solo = len(units) == 1
                nrow = 2 if solo else 34

                def mk_mm_ss(ch):
                    def go():
                        if ch == 0:
                            st8["ss"] = ps_tr.tile([nrow, width], DT_F32,
                                                   tag="tr", name="ss_t")
                        csl = slice(ch * 512, (ch + 1) * 512)
                        nc.tensor.matmul(st8["ss"][:, csl],
                                         lhsT=hselA[:, 0:nrow],
                                         rhs=get_a()[:, csl],
                                         start=True, stop=solo)
                        if not solo:
                            nc.tensor.matmul(st8["ss"][:, csl],
                                             lhsT=hselB[:, 0:nrow],
                                             rhs=get_b()[:, csl],
                                             start=False, stop=True)
                    return go

                def act_ln():
                    lt = ltp.tile([nrow, width], DT_F32, tag="lt", name="lt_t")
                    st8["lt"] = lt
                    nc.scalar.activation(lt, st8["ss"], AF.Ln)

                def act_exp():
                    st = stp.tile([nrow, width], DT_BF, tag="st", name="st_t")
                    st8["st"] = st
                    nc.scalar.activation(st, st8["lt"], AF.Exp,
                                         tb_sb[0:nrow, cfg:cfg + 1], -0.5)

                def mk_bcmul(i):
                    which, mc, u = units[i]
                    dst = (qt if which == "q" else kt)[mc]
                    r0 = 32 * (i % 2)
                    c0 = 512 * (i // 2)
                    sl = slice(u * 512, (u + 1) * 512)

                    def go():
                        rb = ps_tr.tile([128, 512], DT_F32, tag="tr",
                                        name="rb_t")
                        nc.tensor.matmul(
                            rb, lhsT=sel2[r0:r0 + 2, :],
                            rhs=st8["st"][r0:r0 + 2, c0:c0 + 512],
                            start=True, stop=True)
                        nc.vector.tensor_mul(dst[:, sl], dst[:, sl], rb)
                    return go

                steps = [(215, mk_mm_ss(ch), "start" if ch == 0 else "mid")
                         for ch in range(nh)]
                steps.append((450, act_ln, "end"))
                steps.append((450, act_exp, "safe"))
                for i in range(len(units)):
                    steps.append((650, mk_bcmul(i), "safe", units[i]))
                return steps

            def v_unit_steps(m):
                st8 = {}

                def mk_mm(c):
                    def go():
                        if c == 0:
                            st8["vp"] = ps_tr.tile([128, 256], DT_F32,
                                                   tag="tr", name="vp_t")
                        nc.tensor.matmul(
                            st8["vp"],
                            lhsT=xv_sb[:, c, m * 128:(m + 1) * 128],
                            rhs=wv_sb[:, c, :],
                            start=(c == 0),
                            stop=False,
                        )
                    return go

                def bias_drain():
                    nc.tensor.matmul(
                        st8["vp"],
                        lhsT=ones_row[0:1, 0:128],
                        rhs=bv_sb[0:1, :],
                        start=False, stop=True)
                    nc.vector.tensor_copy(
                        out=v_sb[:, m, :, 0:HD],
                        in_=st8["vp"].rearrange("p (h d) -> p h d", h=HPC))

                steps = []
                for c in range(0, KC_E, 2):
                    cc = c
                    steps.append((230,
                                  lambda cc=cc: (mk_mm(cc)(), mk_mm(cc + 1)()),
                                  "start" if c == 0 else "mid"))
                steps.append((600, bias_drain, "end", ("v", m)))
                return steps

            def outproj_steps(m, n2):
                sl_n = slice(n2 * 512, (n2 + 1) * 512)
                st8 = {}

                def mms():
                    st8["op"] = ps_tr.tile([128, 512], DT_F32, tag="tr",
                                           name="op_t")
                    nc.tensor.matmul(st8["op"],
                                     lhsT=heads_t[0][:, m * 128:(m + 1) * 128],
                                     rhs=wo_sb[:, 0, sl_n],
                                     start=True, stop=False)
                    nc.tensor.matmul(st8["op"],
                                     lhsT=heads_t[1][:, m * 128:(m + 1) * 128],
                                     rhs=wo_sb[:, 1, sl_n],
                                     start=False, stop=True)

                def drain():
                    ob = outs.tile([128, 512], DT_BF, tag="ob", name="ob_t")
                    nc.vector.tensor_copy(ob, st8["op"])
                    nc.sync.dma_start(out=outp[m * 128:(m + 1) * 128, sl_n],
                                      in_=ob)
                return [(430, mms, "start"), (700, drain, "end")]

            # =============== filler machinery ============================
            # Steps are (cost_ns, fn, kind): "start" acquires the ps_tr
            # ring, "end" releases it, "mid" holds, "safe" self-contained.
            # Inline code must never alloc ps_tr while a filler unit holds
            # it with its releasing step un-emitted (deadlock), hence
            # flush_unit().
            fillers = deque()
            carry = [0.0]
            tr_open = [False]
            emitted = set()

            def _run_step(step):
                cost, fn, kind = step[0], step[1], step[2]
                fn()
                if kind == "start":
                    tr_open[0] = True
                elif kind == "end":
                    tr_open[0] = False
                if len(step) > 3:
                    emitted.add(step[3])

            def pump(budget_ns):
                budget = budget_ns + carry[0]
                while fillers and budget >= fillers[0][0]:
                    budget -= fillers[0][0]
                    _run_step(fillers.popleft())
                carry[0] = min(budget, 800.0)

            def flush_unit():
                while fillers and tr_open[0]:
                    _run_step(fillers.popleft())

            def gate(label):
                """Emission-order dependency: force-emit filler steps until
                the unit producing `label` has been emitted. Without this, a
                consumer emitted before its producer has no dependency edge
                (Tile resolves deps in emission order) -> races."""
                while label not in emitted:
                    assert fillers, f"gate on {label} but fillers empty"
                    _run_step(fillers.popleft())

            def add_unit_group(units_seq, cfg):
                """Steps for a group of proj units + their joint norm. sq2
                staging tiles are allocated lazily at first use so pool ring
                order matches instruction emission order."""
                steps = []
                n = len(units_seq)
                assert n in (1, 2, 4)
                width = 512 * max(1, n // 2)
                hold = {}

                def getter(key):
                    def get():
                        if key not in hold:
                            hold[key] = sqp.tile([128, width], DT_BF,
                                                 tag="sq2", name=f"sq2{key}")
                        return hold[key]
                    return get

                get_a, get_b = getter("a"), getter("b")
                for i, (which, mc, u) in enumerate(units_seq):
                    tgt = get_a if i % 2 == 0 else get_b
                    steps += proj_unit_steps(which, mc, u, tgt, i // 2)
                steps += norm_group_steps(units_seq, cfg, get_a, get_b, width)
                return steps

            def run_steps(steps):
                for step in steps:
                    _run_step(step)

            # =============== head phase ==================================
            for m in range(2):
                run_steps(v_unit_steps(m))
            run_steps(add_unit_group([("q", 0, 0), ("k", 0, 0)], CFG_MIX0))
            run_steps(add_unit_group([("q", 0, 1), ("k", 0, 1)], CFG_MIX0))
            run_steps(v_unit_steps(2))
            run_steps(v_unit_steps(3))

            # =============== filler stream (need-ordered) ================
            for m in range(4, 8):
                for st_ in v_unit_steps(m):
                    fillers.append(st_)
            for st_ in add_unit_group([("k", 0, 2)], CFG_ALLK0):
                fillers.append(st_)
            for m in range(8, 12):
                for st_ in v_unit_steps(m):
                    fillers.append(st_)
            for st_ in add_unit_group([("k", 0, 3)], CFG_ALLK0):
                fillers.append(st_)
            for m in range(12, MQ):
                for st_ in v_unit_steps(m):
                    fillers.append(st_)
            for st_ in add_unit_group([("q", 0, 2), ("q", 0, 3)], CFG_ALLQ):
                fillers.append(st_)
            for st_ in add_unit_group([("k", 1, 0), ("k", 1, 1)], CFG_ALLK1):
                fillers.append(st_)
            for st_ in add_unit_group([("k", 1, 2), ("k", 1, 3)], CFG_ALLK1):
                fillers.append(st_)
            for st_ in add_unit_group([("q", 1, 0), ("q", 1, 1)], CFG_ALLQ):
                fillers.append(st_)
            for st_ in add_unit_group([("q", 1, 2), ("q", 1, 3)], CFG_ALLQ):
                fillers.append(st_)

            # =============== attention ===================================
            # Software-pipelined by one iteration: at step (qb, kc) the PE
            # queue gets QK(qb,kc) and then PV of the previous step, whose
            # ex tile ACT finished an iteration ago -- the PE never waits
            # on the exp it just fed.  Per-qb softmax normalization is
            # emitted split: the 1/z copies right after the qb's last PV,
            # the broadcast/recip/mul just before the next qb's first PV
            # (WAR on the 2-deep ps_o ring requires normalize reads to be
            # emitted before the slot's next writer).
            def attention_pair(p, budget, after_qb=None):
                pend = [None]     # (qb, kc, ex, o0, o1)
                pnorm = [None]    # (qb, zs0, zs1, o0, o1)

                def emit_pv(t):
                    qb, kc, ex, o0, o1 = t
                    gate(("v", kc))
                    nc.tensor.matmul(
                        o0[0:65, :],
                        lhsT=v_sb[:, kc, 2 * p, :],
                        rhs=ex[:, 0:512],
                        start=(kc == 0), stop=(kc == MQ - 1))
                    nc.tensor.matmul(
                        o1[0:65, :],
                        lhsT=v_sb[:, kc, 2 * p + 1, :],
                        rhs=ex[:, 512:1024],
                        start=(kc == 0), stop=(kc == MQ - 1))

                def emit_zs(t):
                    qb, kc, ex, o0, o1 = t
                    zs0 = zz.tile([1, 512], DT_BF, tag="zi", name="zs0_t")
                    nc.vector.tensor_copy(zs0, o0[64:65, :])
                    zs1 = zz.tile([1, 512], DT_BF, tag="zi", name="zs1_t")
                    nc.vector.tensor_copy(zs1, o1[64:65, :])
                    pnorm[0] = (qb, zs0, zs1, o0, o1)

                def emit_norm():
                    qb, zs0, zs1, o0, o1 = pnorm[0]
                    pnorm[0] = None
                    sl_q = slice(qb * 512, (qb + 1) * 512)
                    zb0 = ps_zb.tile([64, 512], DT_F32, tag="zb",
                                     name="zb0_t")
                    nc.tensor.matmul(zb0, lhsT=ones_row[0:1, 0:64], rhs=zs0,
                                     start=True, stop=True)
                    flush_unit()
                    zb1 = ps_tr.tile([64, 512], DT_F32, tag="tr",
                                     name="zb1_t")
                    nc.tensor.matmul(zb1, lhsT=ones_row[0:1, 0:64], rhs=zs1,
                                     start=True, stop=True)
                    for hl, (o, zb) in enumerate(((o0, zb0), (o1, zb1))):
                        zbi = zz.tile([64, 512], DT_F32, tag="zbi",
                                      name="zbi_t")
                        nc.vector.reciprocal_approx_fast(out=zbi, in_=zb)
                        if hl == 0:
                            nc.vector.tensor_mul(
                                heads_t[p][0:64, sl_q], o[0:64, :], zbi)
                        else:
                            t2 = work.tile([64, 512], DT_BF, tag="t2",
                                           name="t2_t")
                            nc.vector.tensor_mul(t2, o[0:64, :], zbi)
                            nc.sync.dma_start(
                                out=heads_t[p][64:128, sl_q], in_=t2)
                    if after_qb is not None:
                        after_qb(qb)

                for qb in range(4):
                    o0 = ps_o.tile([128, 512], DT_F32, tag="o", name="o0_t")
                    o1 = ps_o.tile([128, 512], DT_F32, tag="o", name="o1_t")
                    for kc in range(MQ):
                        gate(("k", p, kc // 4))
                        gate(("q", p, qb))
                        sc = ps_sc.tile([128, 1024], DT_F32, tag="sc",
                                        name="sc_t")
                        nc.tensor.matmul(
                            sc[:, 0:512],
                            lhsT=kt[p][0:64, kc * 128:(kc + 1) * 128],
                            rhs=qt[p][0:64, qb * 512:(qb + 1) * 512],
                            start=True, stop=True)
                        nc.tensor.matmul(
                            sc[:, 512:1024],
                            lhsT=kt[p][64:128, kc * 128:(kc + 1) * 128],
                            rhs=qt[p][64:128, qb * 512:(qb + 1) * 512],
                            start=True, stop=True)
                        ex = expool.tile([128, 1024], DT_BF, tag="ex",
                                         name="ex_t")
                        nc.scalar.activation(ex, sc, AF.Exp)
                        if pnorm[0] is not None:
                            emit_norm()
                        if pend[0] is not None:
                            emit_pv(pend[0])
                            if pend[0][1] == MQ - 1:
                                emit_zs(pend[0])
                        pend[0] = (qb, kc, ex, o0, o1)
                        pump(budget)
                # tail of this pair: flush pending PV + normalize
                emit_pv(pend[0])
                emit_zs(pend[0])
                pend[0] = None
                emit_norm()

            attention_pair(0, budget=700)

            def pair1_after_qb(qb):
                for m in range(qb * 4, qb * 4 + 4):
                    for n2 in range(2):
                        for st_ in outproj_steps(m, n2):
                            fillers.append(st_)

            attention_pair(1, budget=620, after_qb=pair1_after_qb)

            # =============== tail ========================================
            while fillers:
                _run_step(fillers.popleft())

    nc.compile()
    return nc


_CACHE = {}


def _get_program():
    if "nc" not in _CACHE:
        _CACHE["nc"] = build_program()
    return _CACHE["nc"]


def make_in_maps(query, key, value, in_proj_weight, in_proj_bias,
                 out_proj_weight, out_proj_bias, tau):
    query = np.asarray(query, np.float32)
    key = np.asarray(key, np.float32)
    value = np.asarray(value, np.float32)
    W = np.asarray(in_proj_weight, np.float32)
    bias = np.asarray(in_proj_bias, np.float32)
    Wo = np.asarray(out_proj_weight, np.float32)
    tau_c = np.maximum(np.asarray(tau, np.float32).reshape(H), TAU_MIN)

    xT = {}
    for b in range(B):
        xT["q", b] = np.ascontiguousarray(
            query[:, b, :].T).astype(BF16).reshape(KC_E, 128, S)
        xT["k", b] = np.ascontiguousarray(
            key[:, b, :].T).astype(BF16).reshape(KC_E, 128, S)
        xT["v", b] = np.ascontiguousarray(
            value[:, b, :].T).astype(BF16).reshape(KC_E, 128, S)

    in_maps = []
    for c in range(NCORES):
        b = c // 4
        h0 = HPC * (c % 4)
        rows = slice(h0 * HD, (h0 + HPC) * HD)
        rows_k = slice(E + h0 * HD, E + (h0 + HPC) * HD)
        rows_v = slice(2 * E + h0 * HD, 2 * E + (h0 + HPC) * HD)

        # exp bias configs [34, 4]: ln(1/tau) per row slot.
        # rows {0,1} = "A" slot pair (even, odd head), {32,33} = "B".
        ltau = np.log(1.0 / tau_c)
        tb = np.zeros((34, 4), np.float32)
        tb[32, CFG_MIX0] = ltau[h0 + 0]
        tb[33, CFG_MIX0] = ltau[h0 + 1]
        tb[0, CFG_ALLK0] = ltau[h0 + 0]
        tb[1, CFG_ALLK0] = ltau[h0 + 1]
        tb[32, CFG_ALLK0] = ltau[h0 + 0]
        tb[33, CFG_ALLK0] = ltau[h0 + 1]
        tb[0, CFG_ALLK1] = ltau[h0 + 2]
        tb[1, CFG_ALLK1] = ltau[h0 + 3]
        tb[32, CFG_ALLK1] = ltau[h0 + 2]
        tb[33, CFG_ALLK1] = ltau[h0 + 3]
        # CFG_ALLQ stays zero
        sel2 = np.zeros((34, 128), np.float32)
        for r0 in (0, 32):
            sel2[r0 + 0, 0:64] = 1.0
            sel2[r0 + 1, 64:128] = 1.0

        bq_col = np.zeros((128, 2), np.float32)
        bk_col = np.zeros((128, 2), np.float32)
        for mc in range(NPAIR):
            bq_col[:, mc] = bias[rows][mc * 128:(mc + 1) * 128]
            bk_col[:, mc] = bias[rows_k][mc * 128:(mc + 1) * 128]

        in_maps.append({
            "xq_t": xT["q", b],
            "xk_t": xT["k", b],
            "xv_t": xT["v", b],
            "wq_t": np.ascontiguousarray(
                W[rows, :].T).astype(BF16).reshape(KC_E, 128, 256),
            "wk_t": np.ascontiguousarray(
                W[rows_k, :].T).astype(BF16).reshape(KC_E, 128, 256),
            "wv_t": np.ascontiguousarray(
                W[rows_v, :].T).astype(BF16).reshape(KC_E, 128, 256),
            "bq_col": bq_col,
            "bk_col": bk_col,
            "b_v": bias[rows_v].reshape(1, 256).astype(BF16),
            "wo_t": np.ascontiguousarray(
                Wo[:, rows].T).astype(BF16).reshape(2, 128, E),
            "tb": tb,
            "sel2": sel2.astype(BF16),
        })
    return in_maps


def assemble_out(results, out_proj_bias):
    bo = np.asarray(out_proj_bias, np.float32)
    out = np.zeros((S, B, E), np.float32)
    for c in range(NCORES):
        out[:, c // 4, :] += np.asarray(results[c]["out_p"], np.float32)
    out += bo[None, None, :]
    return out


def kernel(query, key, value, in_proj_weight, in_proj_bias,
           out_proj_weight, out_proj_bias, tau):
    nc = _get_program()
    in_maps = make_in_maps(query, key, value, in_proj_weight, in_proj_bias,
                           out_proj_weight, out_proj_bias, tau)
    res = run_bass_kernel_spmd(nc, in_maps, core_ids=list(range(NCORES)))
    return assemble_out(res.results, out_proj_bias)


if __name__ == "__main__":
    import reference

    inputs = {k: np.asarray(v) for k, v in reference.setup_inputs().items()}
    out = kernel(**inputs)
    print("out shape", out.shape, out.dtype)


# revision 9
# speedup vs baseline: 1.4131x; 1.0154x over previous
"""Cosine multihead attention on 8 Trainium2 NeuronCores.

Sharding: batch*heads across cores. Core c handles batch b = c // 4 and the
4 heads [4*(c%4), 4*(c%4)+4). Each core computes its heads' q/k/v projections
(tensor-parallel slices of in_proj), full attention for its (B,H) slice, and a
partial out-projection (rank-256 contribution). The host sums the 4 partials
per batch and adds out_proj_bias (partials shipped bf16, summed fp32).

Schedule: the attention inner loop is software-pipelined by one iteration —
the PE emits QK(kc) then PV(kc-1), so the exp(kc) latency on ACT is hidden
behind PV(kc-1) + filler work instead of stalling the PE queue (PV(kc)
directly behind QK(kc) exposed the full ACT latency every iteration and let
the PE HAM throttle to 1.2 GHz). Projections and the out-projection are
emitted as PE "filler" steps pumped into the loop at a per-iteration budget.

Key tricks:
- q,k projected in transposed orientation (head_dim on partitions) so QK^T
  needs no transpose; 2 heads per PE pass via row tiling (K=64 at bases 0/64).
- 1/sqrt(sumsq) computed as exp(-0.5*ln(x)) on ACT: Ln and Exp share the
  natural_log_exp_and_others table set (enforced via a table-list override),
  so norms slot into the exp stream with zero table reloads. Sumsq for up to
  4 projection units batches into one [97, W] PSUM tile (row slots 0/32/64/96)
  -> one ln + one exp per group.
- 1/tau folds into the exp's per-partition bias: st = exp(-0.5 ln(ss) +
  ln(1/tau)) directly yields the combined normalizer for the k side.
- all broadcast matmuls (norm factors, softmax 1/z) run in bf16 (fp32 lhsT
  would force the PE into the 2-pass LOW_HIGH fp32 mode, ~4x slower).
- projection bias added via per-partition tensor_scalar_add during the PSUM
  drain; v bias via a prebuilt broadcast tile in the v drain add.
- softmax denominators ride as a 65th v column (ones) through PV.
- inputs DMA'd with 3D access patterns (one trigger per 1MB column block)
  spread across the sync/scalar/vector/gpsimd queues, need-ordered.
"""

import os
import sys

if "/opt/trn_rl_repo" not in sys.path:
    sys.path.insert(0, "/opt/trn_rl_repo")

from collections import deque

import numpy as np
import ml_dtypes

import bass_rust as _bass_rust
import concourse.bass as bass
import concourse.tile as tile
from concourse import bacc, mybir
from concourse.bass_utils import run_bass_kernel_spmd
from concourse.hw_specs import get_activation_tables

S, B, E, H = 2048, 2, 1024, 16
HD = E // H            # 64
HPC = 4                # heads per core
NCORES = 8
TAU_MIN = 0.01

BF16 = ml_dtypes.bfloat16
DT_BF = mybir.dt.bfloat16
DT_F32 = mybir.dt.float32

KC_E = E // 128        # 8 contraction chunks for projections
MQ = S // 128          # 16 seq chunks of 128
NPAIR = HPC // 2       # 2 head pairs per core
AF = mybir.ActivationFunctionType

# exp bias config columns (rows 0/32/64/96 hold per-slot ln(1/tau) or 0)
CFG_MIX0, CFG_ALLK0, CFG_ALLK1, CFG_ALLQ = range(4)


def _insert_act_loads_shared(nc):
    """Drop-in for Bacc.insert_act_table_loads: keep the canonical set list
    (ids must stay aligned with act_info.json) but strip Exp/Ln from every
    set other than natural_log_exp_and_others, which genuinely contains
    both. The placement pass then settles on that one set -> a single
    table load instead of one per Ln<->Exp alternation."""
    tables = []
    for name, funcs in get_activation_tables(nc.m.arch).items():
        if name != "natural_log_exp_and_others":
            funcs = {f for f in funcs if f not in (AF.Exp, AF.Ln)}
        tables.append((name, funcs))
    _bass_rust.insert_act_table_loads(nc, tables)


def build_program():
    nc = bacc.Bacc(None)
    nc.insert_act_table_loads = lambda: _insert_act_loads_shared(nc)

    xq = nc.dram_tensor("xq_t", [KC_E, 128, S], DT_BF, kind="ExternalInput")
    xk = nc.dram_tensor("xk_t", [KC_E, 128, S], DT_BF, kind="ExternalInput")
    xv = nc.dram_tensor("xv_t", [KC_E, 128, S], DT_BF, kind="ExternalInput")
    wq = nc.dram_tensor("wq_t", [KC_E, 128, 256], DT_BF, kind="ExternalInput")
    wk = nc.dram_tensor("wk_t", [KC_E, 128, 256], DT_BF, kind="ExternalInput")
    wv = nc.dram_tensor("wv_t", [KC_E, 128, 256], DT_BF, kind="ExternalInput")
    bqc = nc.dram_tensor("bq_col", [128, 2], DT_F32, kind="ExternalInput")
    bkc = nc.dram_tensor("bk_col", [128, 2], DT_F32, kind="ExternalInput")
    bv = nc.dram_tensor("b_v", [1, 256], DT_BF, kind="ExternalInput")
    wo = nc.dram_tensor("wo_t", [2, 128, E], DT_BF, kind="ExternalInput")
    tb_in = nc.dram_tensor("tb", [34, 4], DT_F32, kind="ExternalInput")
    sel2_in = nc.dram_tensor("sel2", [34, 128], DT_BF, kind="ExternalInput")
    outp = nc.dram_tensor("out_p", [S, E], DT_BF, kind="ExternalOutput")

    with tile.TileContext(nc) as tc:
        with (
            tc.tile_pool(name="consts", bufs=1) as consts,
            tc.tile_pool(name="xin", bufs=1) as xin,
            tc.tile_pool(name="wts", bufs=1) as wts,
            tc.tile_pool(name="qk", bufs=1) as qkpool,
            tc.tile_pool(name="vsb", bufs=1) as vpool,
            tc.tile_pool(name="sqp", bufs=2) as sqp,
            tc.tile_pool(name="stp", bufs=1) as stp,
            tc.tile_pool(name="exp", bufs=14) as expool,
            tc.tile_pool(name="work", bufs=2) as work,
            tc.tile_pool(name="ltp", bufs=1) as ltp,
            tc.tile_pool(name="zz", bufs=2) as zz,
            tc.tile_pool(name="outs", bufs=2) as outs,
            tc.tile_pool(name="ps_sc", bufs=2, space="PSUM") as ps_sc,
            tc.tile_pool(name="ps_o", bufs=2, space="PSUM") as ps_o,
            tc.tile_pool(name="ps_tr", bufs=1, space="PSUM") as ps_tr,
            tc.tile_pool(name="ps_zb", bufs=1, space="PSUM") as ps_zb,
        ):
            # ---- constants ----------------------------------------------
            tb_sb = consts.tile([34, 4], DT_F32, tag="tb")
            nc.sync.dma_start(out=tb_sb, in_=tb_in[:, :])
            bq_sb = consts.tile([128, 2], DT_F32, tag="bq")
            nc.sync.dma_start(out=bq_sb, in_=bqc[:, :])
            bk_sb = consts.tile([128, 2], DT_F32, tag="bk")
            nc.sync.dma_start(out=bk_sb, in_=bkc[:, :])
            bv_sb = consts.tile([1, 256], DT_BF, tag="bv")
            nc.sync.dma_start(out=bv_sb, in_=bv[:, :])
            sel2 = consts.tile([34, 128], DT_BF, tag="sel2")
            nc.sync.dma_start(out=sel2, in_=sel2_in[:, :])

            ones_row = consts.tile([1, 128], DT_BF, tag="ones_row")
            nc.vector.memset(ones_row, 1.0)
            # sumsq selectors: A -> out rows 0 (parts 0:64) / 1 (64:128),
            #                  B -> out rows 32 / 33
            hselA = consts.tile([128, 34], DT_BF, tag="hselA")
            nc.vector.memset(hselA, 0.0)
            nc.vector.memset(hselA[0:64, 0:1], 1.0)
            nc.vector.memset(hselA[64:128, 1:2], 1.0)
            hselB = consts.tile([128, 34], DT_BF, tag="hselB")
            nc.vector.memset(hselB, 0.0)
            nc.vector.memset(hselB[0:64, 32:33], 1.0)
            nc.vector.memset(hselB[64:128, 33:34], 1.0)

            # ---- weights (one 3D DMA each) ------------------------------
            wq_sb = wts.tile([128, KC_E, 256], DT_BF, tag="wq")
            wk_sb = wts.tile([128, KC_E, 256], DT_BF, tag="wk")
            wv_sb = wts.tile([128, KC_E, 256], DT_BF, tag="wv")
            nc.scalar.dma_start(
                out=wk_sb[:, :, :], in_=wk[:, :, :].rearrange("c p n -> p c n"))
            nc.sync.dma_start(
                out=wq_sb[:, :, :], in_=wq[:, :, :].rearrange("c p n -> p c n"))
            nc.gpsimd.dma_start(
                out=wv_sb[:, :, :], in_=wv[:, :, :].rearrange("c p n -> p c n"))

            # ---- activations, need-ordered 512-col blocks spread over the
            # scalar/sync HWDGE rings + the gpsimd SWDGE ring so the first
            # projection units' data lands in ~10us instead of waiting on a
            # single queue's FIFO -------------------------------------------
            xq_sb = xin.tile([128, KC_E, S], DT_BF, tag="xq")
            xk_sb = xin.tile([128, KC_E, S], DT_BF, tag="xk")
            xv_sb = xin.tile([128, KC_E, S], DT_BF, tag="xv")

            def ldx(eng, dst, src, c0, c1):
                eng.dma_start(out=dst[:, :, c0:c1],
                              in_=src[:, :, c0:c1].rearrange("c p s -> p c s"))

            ldx(nc.scalar, xk_sb, xk, 0, 512)
            ldx(nc.sync, xq_sb, xq, 0, 512)
            ldx(nc.gpsimd, xv_sb, xv, 0, 512)
            ldx(nc.scalar, xk_sb, xk, 512, 1024)
            ldx(nc.sync, xq_sb, xq, 512, 1024)
            ldx(nc.gpsimd, xv_sb, xv, 512, 1024)
            ldx(nc.scalar, xk_sb, xk, 1024, 1536)
            ldx(nc.scalar, xk_sb, xk, 1536, 2048)
            ldx(nc.sync, xv_sb, xv, 1024, 1536)
            ldx(nc.gpsimd, xv_sb, xv, 1536, 2048)
            ldx(nc.sync, xq_sb, xq, 1024, 1536)
            ldx(nc.sync, xq_sb, xq, 1536, 2048)
            wo_sb = wts.tile([128, 2, E], DT_BF, tag="wo")
            nc.gpsimd.dma_start(
                out=wo_sb[:, :, :], in_=wo[:, :, :].rearrange("c p n -> p c n"))

            qt = [qkpool.tile([128, S], DT_BF, tag=f"qt{p}", name=f"qt{p}")
                  for p in range(NPAIR)]
            kt = [qkpool.tile([128, S], DT_BF, tag=f"kt{p}", name=f"kt{p}")
                  for p in range(NPAIR)]
            heads_t = [qkpool.tile([128, S], DT_BF, tag=f"ht{p}", name=f"ht{p}")
                       for p in range(NPAIR)]

            v_sb = vpool.tile([128, MQ, HPC, HD + 1], DT_BF, tag="v")
            nc.vector.memset(v_sb[:, :, :, HD:HD + 1], 1.0)

            # =============== unit emitters ===============================
            # A projection "unit" computes a [128, 512] block of qt/kt.
            # Units are (which, mc, u): which 'q'/'k', mc head-pair, u block.

            def proj_unit_steps(which, mc, u, get_sq2, half):
                dst = (qt if which == "q" else kt)[mc]
                w_sb = wq_sb if which == "q" else wk_sb
                x_sb = xq_sb if which == "q" else xk_sb
                b_sb = bq_sb if which == "q" else bk_sb
                sl = slice(u * 512, (u + 1) * 512)
                st8 = {}

                def mk_mm(c):
                    def go():
                        if c == 0:
                            st8["pp"] = ps_tr.tile([128, 512], DT_F32,
                                                   tag="tr", name="pp_t")
                        nc.tensor.matmul(
                            st8["pp"],
                            lhsT=w_sb[:, c, mc * 128:(mc + 1) * 128],
                            rhs=x_sb[:, c, sl],
                            start=(c == 0),
                            stop=(c == KC_E - 1),
                        )
                    return go

                def drain():
                    nc.vector.tensor_scalar_add(
                        dst[:, sl], st8["pp"], b_sb[:, mc:mc + 1])
                    nc.vector.tensor_mul(
                        get_sq2()[:, half * 512:(half + 1) * 512],
                        dst[:, sl], dst[:, sl])

                steps = []
                for c in range(0, KC_E, 2):
                    cc = c
                    steps.append((430,
                                  lambda cc=cc: (mk_mm(cc)(), mk_mm(cc + 1)()),
                                  "start" if c == 0 else "mid"))
                steps.append((900, drain, "end"))
                return steps

            def norm_group_steps(units, cfg, get_a, get_b, width):
                """units: 1, 2 or 4 (which, mc, u). Unit slot i: row pair
                (0,1) for i%2==0 via hselA / (32,33) for i%2==1 via hselB;
                col half i//2. One ln + one exp(bias=ln(1/tau), scale=-0.5)
                per group, then per-unit bf16 broadcast-matmul + normalize
                muls."""
                st8 = {}
                nh = width // 512
                solo = len(units) == 1
                nrow = 2 if solo else 34

                def mk_mm_ss(ch):
                    def go():
                        if ch == 0:
                            st8["ss"] = ps_tr.tile([nrow, width], DT_F32,
                                                   tag="tr", name="ss_t")
                        csl = slice(ch * 512, (ch + 1) * 512)
                        nc.tensor.matmul(st8["ss"][:, csl],
                                         lhsT=hselA[:, 0:nrow],
                                         rhs=get_a()[:, csl],
                                         start=True, stop=solo)
                        if not solo:
                            nc.tensor.matmul(st8["ss"][:, csl],
                                             lhsT=hselB[:, 0:nrow],
                                             rhs=get_b()[:, csl],
                                             start=False, stop=True)
                    return go

                def act_ln():
                    lt = ltp.tile([nrow, width], DT_F32, tag="lt", name="lt_t")
                    st8["lt"] = lt
                    nc.scalar.activation(lt, st8["ss"], AF.Ln)

                def act_exp():
                    st = stp.tile([nrow, width], DT_BF, tag="st", name="st_t")
                    st8["st"] = st
                    nc.scalar.activation(st, st8["lt"], AF.Exp,
                                         tb_sb[0:nrow, cfg:cfg + 1], -0.5)

                def mk_bcmul(i):
                    which, mc, u = units[i]
                    dst = (qt if which == "q" else kt)[mc]
                    r0 = 32 * (i % 2)
                    c0 = 512 * (i // 2)
                    sl = slice(u * 512, (u + 1) * 512)

                    def go():
                        rb = ps_tr.tile([128, 512], DT_F32, tag="tr",
                                        name="rb_t")
                        nc.tensor.matmul(
                            rb, lhsT=sel2[r0:r0 + 2, :],
                            rhs=st8["st"][r0:r0 + 2, c0:c0 + 512],
                            start=True, stop=True)
                        nc.vector.tensor_mul(dst[:, sl], dst[:, sl], rb)
                    return go

                steps = [(215, mk_mm_ss(ch), "start" if ch == 0 else "mid")
                         for ch in range(nh)]
                steps.append((450, act_ln, "end"))
                steps.append((450, act_exp, "safe"))
                for i in range(len(units)):
                    steps.append((650, mk_bcmul(i), "safe", units[i]))
                return steps

            def v_unit_steps(m):
                st8 = {}

                def mk_mm(c):
                    def go():
                        if c == 0:
                            st8["vp"] = ps_tr.tile([128, 256], DT_F32,
                                                   tag="tr", name="vp_t")
                        nc.tensor.matmul(
                            st8["vp"],
                            lhsT=xv_sb[:, c, m * 128:(m + 1) * 128],
                            rhs=wv_sb[:, c, :],
                            start=(c == 0),
                            stop=False,
                        )
                    return go

                def bias_drain():
                    nc.tensor.matmul(
                        st8["vp"],
                        lhsT=ones_row[0:1, 0:128],
                        rhs=bv_sb[0:1, :],
                        start=False, stop=True)
                    nc.vector.tensor_copy(
                        out=v_sb[:, m, :, 0:HD],
                        in_=st8["vp"].rearrange("p (h d) -> p h d", h=HPC))

                steps = []
                for c in range(0, KC_E, 2):
                    cc = c
                    steps.append((230,
                                  lambda cc=cc: (mk_mm(cc)(), mk_mm(cc + 1)()),
                                  "start" if c == 0 else "mid"))
                steps.append((600, bias_drain, "end", ("v", m)))
                return steps

            def outproj_steps(m, n2, pool=None, tag="tr"):
                sl_n = slice(n2 * 512, (n2 + 1) * 512)
                st8 = {}
                use_tr = pool is None

                def mms():
                    pl = ps_tr if use_tr else pool
                    st8["op"] = pl.tile([128, 512], DT_F32, tag=tag,
                                        name="op_t")
                    nc.tensor.matmul(st8["op"],
                                     lhsT=heads_t[0][:, m * 128:(m + 1) * 128],
                                     rhs=wo_sb[:, 0, sl_n],
                                     start=True, stop=False)
                    nc.tensor.matmul(st8["op"],
                                     lhsT=heads_t[1][:, m * 128:(m + 1) * 128],
                                     rhs=wo_sb[:, 1, sl_n],
                                     start=False, stop=True)

                def drain():
                    ob = outs.tile([128, 512], DT_BF, tag="ob", name="ob_t")
                    nc.vector.tensor_copy(ob, st8["op"])
                    nc.sync.dma_start(out=outp[m * 128:(m + 1) * 128, sl_n],
                                      in_=ob)
                return [(800, mms, "start" if use_tr else "safe"),
                        (700, drain, "end" if use_tr else "safe")]

            # =============== filler machinery ============================
            # Steps are (cost_ns, fn, kind): "start" acquires the ps_tr
            # ring, "end" releases it, "mid" holds, "safe" self-contained.
            # Inline code must never alloc ps_tr while a filler unit holds
            # it with its releasing step un-emitted (deadlock), hence
            # flush_unit().
            fillers = deque()
            carry = [0.0]
            tr_open = [False]
            emitted = set()

            def _run_step(step):
                cost, fn, kind = step[0], step[1], step[2]
                fn()
                if kind == "start":
                    tr_open[0] = True
                elif kind == "end":
                    tr_open[0] = False
                if len(step) > 3:
                    emitted.add(step[3])

            def pump(budget_ns):
                budget = budget_ns + carry[0]
                while fillers and budget >= fillers[0][0]:
                    budget -= fillers[0][0]
                    _run_step(fillers.popleft())
                carry[0] = min(budget, 800.0)

            def flush_unit():
                while fillers and tr_open[0]:
                    _run_step(fillers.popleft())

            def gate(label):
                """Emission-order dependency: force-emit filler steps until
                the unit producing `label` has been emitted. Without this, a
                consumer emitted before its producer has no dependency edge
                (Tile resolves deps in emission order) -> races."""
                while label not in emitted:
                    assert fillers, f"gate on {label} but fillers empty"
                    _run_step(fillers.popleft())

            def add_unit_group(units_seq, cfg):
                """Steps for a group of proj units + their joint norm. sq2
                staging tiles are allocated lazily at first use so pool ring
                order matches instruction emission order."""
                steps = []
                n = len(units_seq)
                assert n in (1, 2, 4)
                width = 512 * max(1, n // 2)
                hold = {}

                def getter(key):
                    def get():
                        if key not in hold:
                            hold[key] = sqp.tile([128, width], DT_BF,
                                                 tag="sq2", name=f"sq2{key}")
                        return hold[key]
                    return get

                get_a, get_b = getter("a"), getter("b")
                for i, (which, mc, u) in enumerate(units_seq):
                    tgt = get_a if i % 2 == 0 else get_b
                    steps += proj_unit_steps(which, mc, u, tgt, i // 2)
                steps += norm_group_steps(units_seq, cfg, get_a, get_b, width)
                return steps

            def run_steps(steps):
                for step in steps:
                    _run_step(step)

            # =============== head phase ==================================
            # only the pair-0 u0/u1 projections run inline -- everything
            # else (including the v units) goes through the filler stream
            # so the first QK isn't queued behind DMA-stalled filler MMs
            run_steps(add_unit_group([("q", 0, 0), ("k", 0, 0)], CFG_MIX0))
            run_steps(add_unit_group([("q", 0, 1), ("k", 0, 1)], CFG_MIX0))

            # =============== filler stream (need-ordered) ================
            for m in range(0, 8):
                for st_ in v_unit_steps(m):
                    fillers.append(st_)
            for st_ in add_unit_group([("k", 0, 2)], CFG_ALLK0):
                fillers.append(st_)
            for m in range(8, 12):
                for st_ in v_unit_steps(m):
                    fillers.append(st_)
            for st_ in add_unit_group([("k", 0, 3)], CFG_ALLK0):
                fillers.append(st_)
            for m in range(12, MQ):
                for st_ in v_unit_steps(m):
                    fillers.append(st_)
            for st_ in add_unit_group([("q", 0, 2), ("q", 0, 3)], CFG_ALLQ):
                fillers.append(st_)
            for st_ in add_unit_group([("k", 1, 0), ("k", 1, 1)], CFG_ALLK1):
                fillers.append(st_)
            for st_ in add_unit_group([("k", 1, 2), ("k", 1, 3)], CFG_ALLK1):
                fillers.append(st_)
            for st_ in add_unit_group([("q", 1, 0), ("q", 1, 1)], CFG_ALLQ):
                fillers.append(st_)
            for st_ in add_unit_group([("q", 1, 2), ("q", 1, 3)], CFG_ALLQ):
                fillers.append(st_)

            # =============== attention ===================================
            # Software-pipelined by one iteration: at step (qb, kc) the PE
            # queue gets QK(qb,kc) and then PV of the previous step, whose
            # ex tile ACT finished an iteration ago -- the PE never waits
            # on the exp it just fed.  Per-qb softmax normalization is
            # emitted split: the 1/z copies right after the qb's last PV,
            # the broadcast/recip/mul just before the next qb's first PV
            # (WAR on the 2-deep ps_o ring requires normalize reads to be
            # emitted before the slot's next writer).
            def attention_pair(p, budget, after_qb=None):
                pend = [None]     # (qb, kc, ex, o0, o1)
                pnorm = [None]    # (qb, zs0, zs1, o0, o1)

                def emit_pv(t):
                    qb, kc, ex, o0, o1 = t
                    gate(("v", kc))
                    nc.tensor.matmul(
                        o0[0:65, :],
                        lhsT=v_sb[:, kc, 2 * p, :],
                        rhs=ex[:, 0:512],
                        start=(kc == 0), stop=(kc == MQ - 1))
                    nc.tensor.matmul(
                        o1[0:65, :],
                        lhsT=v_sb[:, kc, 2 * p + 1, :],
                        rhs=ex[:, 512:1024],
                        start=(kc == 0), stop=(kc == MQ - 1))

                def emit_zs(t):
                    qb, kc, ex, o0, o1 = t
                    zs0 = zz.tile([1, 512], DT_BF, tag="zi", name="zs0_t")
                    nc.vector.tensor_copy(zs0, o0[64:65, :])
                    zs1 = zz.tile([1, 512], DT_BF, tag="zi", name="zs1_t")
                    nc.vector.tensor_copy(zs1, o1[64:65, :])
                    pnorm[0] = (qb, zs0, zs1, o0, o1)

                def emit_norm():
                    qb, zs0, zs1, o0, o1 = pnorm[0]
                    pnorm[0] = None
                    sl_q = slice(qb * 512, (qb + 1) * 512)
                    zb0 = ps_zb.tile([64, 512], DT_F32, tag="zb",
                                     name="zb0_t")
                    nc.tensor.matmul(zb0, lhsT=ones_row[0:1, 0:64], rhs=zs0,
                                     start=True, stop=True)
                    flush_unit()
                    zb1 = ps_tr.tile([64, 512], DT_F32, tag="tr",
                                     name="zb1_t")
                    nc.tensor.matmul(zb1, lhsT=ones_row[0:1, 0:64], rhs=zs1,
                                     start=True, stop=True)
                    for hl, (o, zb) in enumerate(((o0, zb0), (o1, zb1))):
                        zbi = zz.tile([64, 512], DT_F32, tag="zbi",
                                      name="zbi_t")
                        nc.vector.reciprocal_approx_fast(out=zbi, in_=zb)
                        if hl == 0:
                            nc.vector.tensor_mul(
                                heads_t[p][0:64, sl_q], o[0:64, :], zbi)
                        else:
                            t2 = work.tile([64, 512], DT_BF, tag="t2",
                                           name="t2_t")
                            nc.vector.tensor_mul(t2, o[0:64, :], zbi)
                            nc.sync.dma_start(
                                out=heads_t[p][64:128, sl_q], in_=t2)
                    if after_qb is not None:
                        after_qb(qb)

                for qb in range(4):
                    o0 = ps_o.tile([128, 512], DT_F32, tag="o", name="o0_t")
                    o1 = ps_o.tile([128, 512], DT_F32, tag="o", name="o1_t")
                    for kc in range(MQ):
                        gate(("k", p, kc // 4))
                        gate(("q", p, qb))
                        sc = ps_sc.tile([128, 1024], DT_F32, tag="sc",
                                        name="sc_t")
                        nc.tensor.matmul(
                            sc[:, 0:512],
                            lhsT=kt[p][0:64, kc * 128:(kc + 1) * 128],
                            rhs=qt[p][0:64, qb * 512:(qb + 1) * 512],
                            start=True, stop=True)
                        nc.tensor.matmul(
                            sc[:, 512:1024],
                            lhsT=kt[p][64:128, kc * 128:(kc + 1) * 128],
                            rhs=qt[p][64:128, qb * 512:(qb + 1) * 512],
                            start=True, stop=True)
                        ex = expool.tile([128, 1024], DT_BF, tag="ex",
                                         name="ex_t")
                        nc.scalar.activation(ex, sc, AF.Exp)
                        if pnorm[0] is not None:
                            emit_norm()
                        if pend[0] is not None:
                            emit_pv(pend[0])
                            if pend[0][1] == MQ - 1:
                                emit_zs(pend[0])
                        pend[0] = (qb, kc, ex, o0, o1)
                        pump(budget)
                # tail of this pair: flush pending PV + normalize
                emit_pv(pend[0])
                emit_zs(pend[0])
                pend[0] = None
                emit_norm()

            attention_pair(0, budget=1000)

            def pair1_after_qb(qb):
                for m in range(qb * 4, qb * 4 + 4):
                    for n2 in range(2):
                        # the last qb's out-projection drains in the kernel
                        # tail, where the attention sc ring (4 banks) is
                        # dead -- use it as a 2-deep ring there instead of
                        # serializing on the 1-deep ps_tr
                        if qb == 3:
                            for st_ in outproj_steps(m, n2, pool=ps_sc,
                                                     tag="sc"):
                                fillers.append(st_)
                        else:
                            for st_ in outproj_steps(m, n2):
                                fillers.append(st_)

            attention_pair(1, budget=700, after_qb=pair1_after_qb)

            # =============== tail ========================================
            while fillers:
                _run_step(fillers.popleft())

    nc.compile()
    return nc


_CACHE = {}


def _get_program():
    if "nc" not in _CACHE:
        _CACHE["nc"] = build_program()
    return _CACHE["nc"]


def make_in_maps(query, key, value, in_proj_weight, in_proj_bias,
                 out_proj_weight, out_proj_bias, tau):
    query = np.asarray(query, np.float32)
    key = np.asarray(key, np.float32)
    value = np.asarray(value, np.float32)
    W = np.asarray(in_proj_weight, np.float32)
    bias = np.asarray(in_proj_bias, np.float32)
    Wo = np.asarray(out_proj_weight, np.float32)
    tau_c = np.maximum(np.asarray(tau, np.float32).reshape(H), TAU_MIN)

    xT = {}
    for b in range(B):
        xT["q", b] = np.ascontiguousarray(
            query[:, b, :].T).astype(BF16).reshape(KC_E, 128, S)
        xT["k", b] = np.ascontiguousarray(
            key[:, b, :].T).astype(BF16).reshape(KC_E, 128, S)
        xT["v", b] = np.ascontiguousarray(
            value[:, b, :].T).astype(BF16).reshape(KC_E, 128, S)

    in_maps = []
    for c in range(NCORES):
        b = c // 4
        h0 = HPC * (c % 4)
        rows = slice(h0 * HD, (h0 + HPC) * HD)
        rows_k = slice(E + h0 * HD, E + (h0 + HPC) * HD)
        rows_v = slice(2 * E + h0 * HD, 2 * E + (h0 + HPC) * HD)

        # exp bias configs [34, 4]: ln(1/tau) per row slot.
        # rows {0,1} = "A" slot pair (even, odd head), {32,33} = "B".
        ltau = np.log(1.0 / tau_c)
        tb = np.zeros((34, 4), np.float32)
        tb[32, CFG_MIX0] = ltau[h0 + 0]
        tb[33, CFG_MIX0] = ltau[h0 + 1]
        tb[0, CFG_ALLK0] = ltau[h0 + 0]
        tb[1, CFG_ALLK0] = ltau[h0 + 1]
        tb[32, CFG_ALLK0] = ltau[h0 + 0]
        tb[33, CFG_ALLK0] = ltau[h0 + 1]
        tb[0, CFG_ALLK1] = ltau[h0 + 2]
        tb[1, CFG_ALLK1] = ltau[h0 + 3]
        tb[32, CFG_ALLK1] = ltau[h0 + 2]
        tb[33, CFG_ALLK1] = ltau[h0 + 3]
        # CFG_ALLQ stays zero
        sel2 = np.zeros((34, 128), np.float32)
        for r0 in (0, 32):
            sel2[r0 + 0, 0:64] = 1.0
            sel2[r0 + 1, 64:128] = 1.0

        bq_col = np.zeros((128, 2), np.float32)
        bk_col = np.zeros((128, 2), np.float32)
        for mc in range(NPAIR):
            bq_col[:, mc] = bias[rows][mc * 128:(mc + 1) * 128]
            bk_col[:, mc] = bias[rows_k][mc * 128:(mc + 1) * 128]

        in_maps.append({
            "xq_t": xT["q", b],
            "xk_t": xT["k", b],
            "xv_t": xT["v", b],
            "wq_t": np.ascontiguousarray(
                W[rows, :].T).astype(BF16).reshape(KC_E, 128, 256),
            "wk_t": np.ascontiguousarray(
                W[rows_k, :].T).astype(BF16).reshape(KC_E, 128, 256),
            "wv_t": np.ascontiguousarray(
                W[rows_v, :].T).astype(BF16).reshape(KC_E, 128, 256),
            "bq_col": bq_col,
            "bk_col": bk_col,
            "b_v": bias[rows_v].reshape(1, 256).astype(BF16),
            "wo_t": np.ascontiguousarray(
                Wo[:, rows].T).astype(BF16).reshape(2, 128, E),
            "tb": tb,
            "sel2": sel2.astype(BF16),
        })
    return in_maps


def assemble_out(results, out_proj_bias):
    bo = np.asarray(out_proj_bias, np.float32)
    out = np.zeros((S, B, E), np.float32)
    for c in range(NCORES):
        out[:, c // 4, :] += np.asarray(results[c]["out_p"], np.float32)
    out += bo[None, None, :]
    return out


def kernel(query, key, value, in_proj_weight, in_proj_bias,
           out_proj_weight, out_proj_bias, tau):
    nc = _get_program()
    in_maps = make_in_maps(query, key, value, in_proj_weight, in_proj_bias,
                           out_proj_weight, out_proj_bias, tau)
    res = run_bass_kernel_spmd(nc, in_maps, core_ids=list(range(NCORES)))
    return assemble_out(res.results, out_proj_bias)


if __name__ == "__main__":
    import reference

    inputs = {k: np.asarray(v) for k, v in reference.setup_inputs().items()}
    out = kernel(**inputs)
    print("out shape", out.shape, out.dtype)
